# revision 19
# baseline (speedup 1.0000x reference)
"""Trainium2 Bass kernel: RMSNorm + QKV + YaRN RoPE + sliding-window GQA attention
with sink logits + output projection + residual.

Sharding: data-parallel over batch (2) x tensor-parallel over KV-head pairs (4).
Each of the 8 cores computes, for one batch element and 2 of the 8 KV heads
(16 of the 64 Q heads), the fused block and a partial output projection.
The host sums the 4 partial projections per batch and adds bias + residual.

v3 numerics/layout:
  - RMSNorm folded into the host prep (t = x * rsqrt(mean(x^2)+eps); norm_w
    folded into qkv_w), so the device consumes pre-normalized activations.
  - QKV + output projection run as fp8e4m3 hi/lo-split matmuls evaluated as
    3 of 4 cross terms via DoubleRow perf mode (0.75 cycles/row, ~0.1% exact).
  - Bias enters via a padded ones-row of t (hidden idx 2880) and a matching
    qkv_w column.
  - RoPE/QK^T/PV run in bf16; probs computed exp(logit - 6ln2) in bf16.
  - Attention output requantized to fp8 hi/lo (scale 32, folded into the
    1/32 "ones" column of vt used for the softmax denominator).
  - Causal + sliding-window masks injected with fp8 DoubleRow matmuls.
"""

import numpy as np
import ml_dtypes

import concourse.bass as bass
import concourse.tile as tile
from concourse import bacc, mybir
from concourse.bass_utils import run_bass_kernel_spmd

# problem constants
B, SEQ, HID = 2, 1024, 2880
NH, NKV, D = 64, 8, 64
KT = 24                 # hidden k-tiles after pad (3072 = 2880 + bias row + pad)
KP = KT // 2            # 12 doublerow k-tile pairs
HIDP = KT * 128
QKV_M = 10              # 1280 rows per core / 128
OUT_M = 23              # output hidden tiles (2944)
OUT_F = 8               # 1024 attn features / 128
NT = SEQ
CH = 512
EPS = 1e-5
MASK_NEG = -96.0        # exact in e4m3
LOGIT_SHIFT = -4.1588830833596715  # -6*ln2
T_SCALE = 8.0
W_SCALE = 256.0
A_SCALE = 32.0
DEQ = 1.0 / (T_SCALE * W_SCALE)
ODEQ = 1.0 / (W_SCALE * A_SCALE)

F32 = mybir.dt.float32
BF = mybir.dt.bfloat16
F8 = mybir.dt.float8e4
NF8 = ml_dtypes.float8_e4m3
NBF = ml_dtypes.bfloat16
DR = mybir.MatmulPerfMode.DoubleRow

PAIR_SWAP = [i ^ 1 for i in range(32)]

# debug bisect: 0=full, 1=phase A only, 2=A+attention (no outproj)
DEBUG_LEVEL = 0


# ---------------------------------------------------------------- device code
def build_nc(reps=1, timing_mode=False):
    nc = bacc.Bacc("TRN2", target_bir_lowering=False, debug=False)
    mult = mybir.AluOpType.mult
    sub = mybir.AluOpType.subtract

    big = "Internal" if timing_mode else "ExternalInput"
    t8_d = nc.dram_tensor("t8", [2 * KT, 128, NT], F8, kind=big)
    wqkv_d = nc.dram_tensor("wqkv", [QKV_M, 128, 2 * KT * 128], F8, kind=big)
    wout_d = nc.dram_tensor("wout", [OUT_M, 128, 2 * OUT_F * 128], F8, kind=big)
    cos_d = nc.dram_tensor("cos128", [128, NT], BF, kind="ExternalInput")
    sin_d = nc.dram_tensor("sin128", [128, NT], BF, kind="ExternalInput")
    maskt_d = nc.dram_tensor("maskt", [128, 512], F8, kind="ExternalInput")
    identb_d = nc.dram_tensor("identb", [128, 1024], F8, kind="ExternalInput")
    sink_d = nc.dram_tensor("sinkrow", [4, CH], F32, kind="ExternalInput")
    id_d = nc.dram_tensor("ident64", [128, 64], F32, kind="ExternalInput")
    out_d = nc.dram_tensor("out_t", [OUT_M, 128, NT], BF,
                           kind="Internal" if timing_mode else "ExternalOutput")
    dummy_d = (nc.dram_tensor("timing_out", [1, 2], F32, kind="ExternalOutput")
               if timing_mode else None)

    with tile.TileContext(nc) as tc:
      for rep in range(reps):
            with tc.tile_pool(name="singles", bufs=1) as singles:
                cos_sb = singles.tile([128, NT], BF)
                sin_sb = singles.tile([128, NT], BF)
                sink_sb = singles.tile([97, CH], F32)
                maskt_sb = singles.tile([128, 512], F8)
                identb_sb = singles.tile([128, 1024], F8)
                id_sb = singles.tile([128, 64], F32)
                shift_sb = singles.tile([128, 1], F32)

                q_sb = singles.tile([128, 8 * NT], BF)
                k_sb = singles.tile([128, NT], BF)
                vt_sb = singles.tile([128, 16 * 65], BF)
                a32_sb = singles.tile([128, 8 * NT], BF)
                ahi_sb = singles.tile([128, 8 * NT], F8)
                alo_sb = singles.tile([128, 8 * NT], F8)

                # ---------------- phase A: qkv (fp8 hi/lo doublerow) + rope
                with (
                    tc.tile_pool(name="tp", bufs=1) as tp,
                    tc.tile_pool(name="wqp", bufs=3) as wqp,
                    tc.tile_pool(name="ropep", bufs=3) as ropep,
                    tc.tile_pool(name="vp", bufs=1) as vp,
                    tc.tile_pool(name="ps_mm", bufs=3, space="PSUM", side="right") as ps_mm,
                ):
                    t_sb = tp.tile([128, 2 * KT * NT], F8)
                    t_v = t_sb.rearrange("p (hl k n) -> p hl k n", hl=2, n=NT)

                    wq_tiles = {}

                    def fetch_wq(m, split=False):
                        wq_tiles[m] = wqp.tile([128, 2 * KT * 128], F8,
                                               tag="wq", name=f"wq{m}")
                        if not split:
                            nc.scalar.dma_start(out=wq_tiles[m], in_=wqkv_d[m, :, :])
                            return
                        wv = wq_tiles[m].rearrange(
                            "p (hl k c) -> p hl k c", hl=2, c=128)
                        for i in range(KP):
                            for hl in range(2):
                                nc.scalar.dma_start(
                                    out=wv[:, hl, 2 * i:2 * i + 2, :],
                                    in_=wqkv_d[m, :,
                                               hl * KT * 128 + i * 256:
                                               hl * KT * 128 + (i + 1) * 256],
                                )

                    fetch_wq(8, split=True)
                    # t stream: per kpair, hi then lo (2D DMAs: dram [p,n])
                    for i in range(KP):
                        for k in (2 * i, 2 * i + 1):
                            nc.sync.dma_start(
                                out=t_v[:, 0, k, :], in_=t8_d[k, :, :])
                        for k in (2 * i, 2 * i + 1):
                            nc.sync.dma_start(
                                out=t_v[:, 1, k, :], in_=t8_d[KT + k, :, :])

                    nc.scalar.dma_start(out=cos_sb, in_=cos_d[:, :])
                    nc.scalar.dma_start(out=sin_sb, in_=sin_d[:, :])
                    nc.scalar.dma_start(out=maskt_sb, in_=maskt_d[:, :])
                    nc.scalar.dma_start(out=identb_sb, in_=identb_d[:, :])
                    nc.scalar.dma_start(out=id_sb, in_=id_d[:, :])
                    for i in range(4):
                        nc.scalar.dma_start(
                            out=sink_sb[32 * i:32 * i + 1, :], in_=sink_d[i:i + 1, :]
                        )
                    nc.vector.memset(shift_sb, LOGIT_SHIFT)
                    nc.vector.memset(vt_sb, 1.0 / A_SCALE)

                    def qkv_mms(m):
                        w_v = wq_tiles.pop(m).rearrange(
                            "p (hl k c) -> p hl k c", hl=2, c=128)
                        ps = ps_mm.tile([128, NT], F32, tag="mm", name=f"mm{m}")
                        for kp in range(KP):
                            ksl = slice(2 * kp, 2 * kp + 2)
                            terms = ((0, 0), (0, 1), (1, 0))  # (w_hl, t_hl)
                            for ti, (whl, thl) in enumerate(terms):
                                for c in range(2):
                                    csl = slice(c * CH, (c + 1) * CH)
                                    nc.tensor.matmul(
                                        ps[:, csl],
                                        w_v[:, whl, ksl, :],
                                        t_v[:, thl, ksl, csl],
                                        start=(kp == 0 and ti == 0),
                                        stop=(kp == KP - 1 and ti == 2),
                                        perf_mode=DR,
                                    )
                        return ps

                    add = mybir.AluOpType.add

                    def rope(dest):
                        # dest: bf16 [128, NT] slice, already dequantized
                        sh = ropep.tile([128, NT], BF, tag="rope")
                        nc.vector.stream_shuffle(out=sh, in_=dest, mask=PAIR_SWAP)
                        nc.vector.tensor_tensor(out=dest, in0=dest, in1=cos_sb, op=mult)
                        nc.vector.tensor_tensor(out=sh, in0=sh, in1=sin_sb, op=mult)
                        nc.vector.tensor_tensor(out=dest, in0=dest, in1=sh, op=add)

                    # k first, then v, then q 0..7
                    fetch_wq(9)
                    fetch_wq(0)
                    ps_k = qkv_mms(8)
                    nc.scalar.activation(
                        out=k_sb, in_=ps_k,
                        func=mybir.ActivationFunctionType.Copy, scale=DEQ)
                    rope(k_sb)

                    fetch_wq(1)
                    ps_v = qkv_mms(9)
                    v_bf = vp.tile([128, NT], F32)
                    nc.scalar.activation(
                        out=v_bf, in_=ps_v,
                        func=mybir.ActivationFunctionType.Copy, scale=DEQ)
                    # transpose v into vt (k-tokens on partitions), batched psum
                    vt_ctx = tc.tile_pool(name="ps_vt", bufs=1, space="PSUM")
                    ps_vt = vt_ctx.__enter__()
                    pst = ps_vt.tile([128, 16 * 64], F32, tag="vt")
                    for g in range(2):
                        for kt in range(8):
                            nc.tensor.matmul(
                                pst[:, (g * 8 + kt) * 64:(g * 8 + kt + 1) * 64],
                                v_bf[g * 64:(g + 1) * 64, kt * 128:(kt + 1) * 128],
                                id_sb[g * 64:(g + 1) * 64, :],
                                is_transpose=True,
                                start=True, stop=True,
                            )
                    for s in range(16):
                        nc.vector.tensor_copy(
                            out=vt_sb[:, s * 65:s * 65 + 64],
                            in_=pst[:, s * 64:(s + 1) * 64],
                        )
                    vt_ctx.__exit__(None, None, None)

                    def q_epilogue(m, ps):
                        dest = q_sb[:, m * NT:(m + 1) * NT]
                        nc.scalar.activation(
                            out=dest, in_=ps,
                            func=mybir.ActivationFunctionType.Copy, scale=DEQ)
                        rope(dest)

                    for m in range(8):
                        if m + 2 < 8:
                            fetch_wq(m + 2)
                        ps = qkv_mms(m)
                        q_epilogue(m, ps)

                # ---------------- phase B: attention + outproj (fp8 hi/lo DR)
                with (
                    tc.tile_pool(name="wop", bufs=3) as wop,
                    tc.tile_pool(name="wexp", bufs=6) as wexp,
                    tc.tile_pool(name="dnp", bufs=6) as dnp,
                    tc.tile_pool(name="otp", bufs=4) as otp,
                    tc.tile_pool(name="ps_att", bufs=3, space="PSUM") as ps_att,
                    tc.tile_pool(name="ps_pv", bufs=3, space="PSUM") as ps_pv,
                    tc.tile_pool(name="ps_o", bufs=2, space="PSUM") as ps_o,
                ):
                    q_v = q_sb.rearrange("p (h t) -> p h t", t=NT)
                    a_v = a32_sb.rearrange("p (h t) -> p h t", t=NT)
                    ahi_v = ahi_sb.rearrange("p (h t) -> p h t", t=NT)
                    alo_v = alo_sb.rearrange("p (h t) -> p h t", t=NT)
                    maskt_v = maskt_sb.rearrange("p (s x c) -> p s x c", s=2, x=2)
                    identb_v = identb_sb.rearrange("p (x c) -> p x c", c=512)

                    def attn_group(qt, a, g):
                        prng = slice(g * 64, (g + 1) * 64)
                        kts = [qt] if qt == 0 else [qt - 1, qt]
                        rhs_q = q_v[prng, 4 * a:4 * a + 4, qt * 128:(qt + 1) * 128]
                        ws = []
                        for kt in kts:
                            psl = ps_att.tile([128, CH], F32, tag="l")
                            sel = 0 if kt == qt else 1
                            nc.tensor.matmul(
                                psl,
                                maskt_v[:, sel, :, :],
                                identb_v[:, 0:2, :],
                                start=True, stop=False,
                                perf_mode=DR,
                            )
                            nc.tensor.matmul(
                                psl,
                                k_sb[prng, kt * 128:(kt + 1) * 128],
                                rhs_q,
                                start=False, stop=True,
                            )
                            w = wexp.tile([128, CH], BF, tag="w")
                            nc.scalar.activation(
                                out=w, in_=psl, func=mybir.ActivationFunctionType.Exp,
                                bias=shift_sb[:, 0:1], scale=1.0,
                            )
                            ws.append((kt, w))
                        pspv = ps_pv.tile([65, CH], F32, tag="pv")
                        for i, (kt, w) in enumerate(ws):
                            nc.tensor.matmul(
                                pspv,
                                vt_sb[:, (g * 8 + kt) * 65:(g * 8 + kt + 1) * 65],
                                w,
                                start=(i == 0),
                                stop=(i == len(ws) - 1),
                            )
                        dn = dnp.tile([1, CH], F32, tag="dn")
                        so = 32 * (2 * g + a)
                        nc.vector.tensor_tensor(
                            out=dn, in0=pspv[64:65, :], in1=sink_sb[so:so + 1, :],
                            op=mybir.AluOpType.add)
                        nc.vector.reciprocal(out=dn, in_=dn)
                        dnb = dnp.tile([64, CH], F32, tag="dnb")
                        nc.gpsimd.partition_broadcast(dnb, dn)
                        asl = (prng, slice(4 * a, 4 * a + 4),
                               slice(qt * 128, (qt + 1) * 128))
                        nc.vector.tensor_tensor(
                            out=a_v[asl], in0=pspv[0:64, :], in1=dnb, op=mult)
                        nc.scalar.activation(
                            out=ahi_v[asl], in_=a_v[asl],
                            func=mybir.ActivationFunctionType.Copy, scale=1.0)
                        nc.vector.tensor_tensor(
                            out=alo_v[asl], in0=a_v[asl], in1=ahi_v[asl], op=sub)

                    def outproj_m(c, m):
                        wo_sb = wop.tile([128, 2 * OUT_F * 128], F8, tag="wo",
                                         name=f"wo{c}_{m}")
                        nc.scalar.dma_start(out=wo_sb, in_=wout_d[m, :, :])
                        wo_v = wo_sb.rearrange("p (hl f x) -> p hl f x", hl=2, x=128)
                        ps = ps_o.tile([128, CH], F32, tag="o", name=f"o{c}_{m}")
                        tsl = slice(c * CH, (c + 1) * CH)
                        for fp in range(OUT_F // 2):
                            fsl = slice(2 * fp, 2 * fp + 2)
                            for ti, (whl, ahl) in enumerate(
                                    ((0, 0), (0, 1), (1, 0))):
                                src = ahi_v if ahl == 0 else alo_v
                                nc.tensor.matmul(
                                    ps,
                                    wo_v[:, whl, fsl, :],
                                    src[:, fsl, tsl],
                                    start=(fp == 0 and ti == 0),
                                    stop=(fp == OUT_F // 2 - 1 and ti == 2),
                                    perf_mode=DR,
                                )
                        ot = otp.tile([128, CH], BF, tag="ot")
                        nc.scalar.activation(
                            out=ot, in_=ps,
                            func=mybir.ActivationFunctionType.Copy, scale=ODEQ)
                        nc.sync.dma_start(
                            out=out_d[m, :, c * CH:(c + 1) * CH], in_=ot
                        )

                    if DEBUG_LEVEL == 1:
                        for mm2 in range(8):
                            ot = otp.tile([128, NT], BF, tag="ot")
                            nc.vector.tensor_copy(
                                out=ot, in_=q_sb[:, mm2 * NT:(mm2 + 1) * NT])
                            nc.sync.dma_start(out=out_d[mm2, :, :], in_=ot)
                    elif DEBUG_LEVEL == 2:
                        for qt in range(8):
                            for a in range(2):
                                for g in range(2):
                                    attn_group(qt, a, g)
                        for mm2 in range(8):
                            ot = otp.tile([128, NT], BF, tag="ot")
                            nc.vector.tensor_copy(
                                out=ot, in_=a32_sb[:, mm2 * NT:(mm2 + 1) * NT])
                            nc.sync.dma_start(out=out_d[mm2, :, :], in_=ot)
                    else:
                        for qt in range(4):
                            for a in range(2):
                                for g in range(2):
                                    attn_group(qt, a, g)
                        groups47 = [(qt, a, g) for qt in range(4, 8)
                                    for a in range(2) for g in range(2)]
                        for m in range(OUT_M):
                            outproj_m(0, m)
                            if m < len(groups47):
                                attn_group(*groups47[m])
                        for m in range(OUT_M):
                            outproj_m(1, m)

      if timing_mode:
          with tc.tile_pool(name="dummyp", bufs=1) as dummyp:
              dt_sb = dummyp.tile([1, 2], F32)
              nc.vector.memset(dt_sb, 1.0)
              nc.sync.dma_start(out=dummy_d[:, :], in_=dt_sb)

    nc.compile()
    return nc


# ---------------------------------------------------------------- host prep
def _rope_tables():
    # verbatim fp32 port of the reference YaRN cache
    steps = np.arange(0, 64, 2, dtype=np.float32)
    freq = np.power(np.float32(150000.0), steps / np.float32(64))
    conc = np.float32(0.1) * np.log(np.float32(32.0)) + 1.0
    d_half = np.float32(32.0)
    log_base = np.log(np.float32(150000.0))
    low = d_half * np.log(np.float32(4096) / (np.float32(32.0) * np.float32(2.0 * np.pi))) / log_base
    high = d_half * np.log(np.float32(4096) / (np.float32(1.0) * np.float32(2.0 * np.pi))) / log_base
    ramp = (np.arange(32, dtype=np.float32) - low) / (high - low)
    mask = 1.0 - np.clip(ramp, 0.0, 1.0)
    inv_freq = (1.0 / (np.float32(32.0) * freq)) * (1.0 - mask) + (1.0 / freq) * mask
    pos = np.arange(SEQ, dtype=np.float32)
    freqs = np.einsum("i,j->ij", pos, inv_freq.astype(np.float32))
    cos = (np.cos(freqs) * conc).astype(np.float32)  # (SEQ, 32)
    sin = (np.sin(freqs) * conc).astype(np.float32)
    return cos, sin


_ILV = np.empty(64, np.int64)
_ILV[0::2] = np.arange(32)
_ILV[1::2] = np.arange(32) + 32


def _hilo(w):
    hi = np.asarray(w, np.float32).astype(NF8)
    lo = (np.asarray(w, np.float32) - hi.astype(np.float32)).astype(NF8)
    return hi, lo


def prep_inputs(x, norm_w, qkv_w, qkv_b, out_w, sinks):
    x = np.asarray(x, np.float32)
    norm_w = np.asarray(norm_w, np.float32)
    qkv_w = np.asarray(qkv_w, np.float32)
    qkv_b = np.asarray(qkv_b, np.float32)
    out_w = np.asarray(out_w, np.float32)
    sinks = np.asarray(sinks, np.float32)

    cos, sin = _rope_tables()
    cosT, sinT = cos.T, sin.T                      # (32, SEQ)
    cos64 = np.repeat(cosT, 2, axis=0)             # lo/hi both use cos_i
    sin64 = np.repeat(sinT, 2, axis=0).copy()
    sin64[0::2] *= -1.0                            # lo gets -sin
    cos128 = np.ascontiguousarray(
        np.concatenate([cos64, cos64], axis=0)).astype(NBF)
    sin128 = np.ascontiguousarray(
        np.concatenate([sin64, sin64], axis=0)).astype(NBF)

    i = np.arange(128)[:, None]
    j = np.arange(128)[None, :]
    maskd = np.where(i <= j, 0.0, MASK_NEG).astype(np.float32)
    maskl = np.where(i > j, 0.0, MASK_NEG).astype(np.float32)
    zz = np.zeros((128, 128), np.float32)
    maskt = np.concatenate([maskd.T, zz, maskl.T, zz], axis=1).astype(NF8)
    identb = np.concatenate(
        [np.tile(np.eye(128, dtype=np.float32), (1, 4)),
         np.zeros((128, 512), np.float32)], axis=1).astype(NF8)

    eye = np.eye(64, dtype=np.float32)
    ident64 = np.ascontiguousarray(np.concatenate([eye, eye], axis=0))

    # host rmsnorm (fp32, matching reference epsilon placement)
    rms = np.mean(x.astype(np.float32) ** 2, axis=-1, keepdims=True)
    t_full = x * (1.0 / np.sqrt(rms + EPS))        # (B, S, H)

    w_eff = qkv_w * norm_w[None, :] * W_SCALE
    b_eff = qkv_b * W_SCALE
    w_eff[:NH * D] *= 0.125
    b_eff[:NH * D] *= 0.125

    in_maps = []
    for c in range(8):
        b, g2 = divmod(c, 4)
        qheads = np.empty(16, np.int64)
        qheads[0::2] = 16 * g2 + np.arange(8)        # g=0 heads, even slots
        qheads[1::2] = 16 * g2 + 8 + np.arange(8)    # g=1 heads, odd slots
        qrows = (qheads[:, None] * D + _ILV[None, :]).reshape(-1)
        krows = NH * D + np.arange(2 * g2 * D, 2 * (g2 + 1) * D)
        vrows = (NH + NKV) * D + np.arange(2 * g2 * D, 2 * (g2 + 1) * D)
        krows = krows.reshape(2, 64)[:, _ILV].reshape(-1)
        rowsel = np.concatenate([qrows, krows, vrows])
        Wc = np.zeros((1280, HIDP), np.float32)
        Wc[:, :HID] = w_eff[rowsel]
        Wc[:, HID] = b_eff[rowsel]                   # bias column (t row = T_SCALE)
        whi, wlo = _hilo(Wc.T)                       # (HIDP, 1280) fp8
        wqkv = np.ascontiguousarray(
            np.stack([whi, wlo])                     # (2, HIDP, 1280)
            .reshape(2, KT, 128, QKV_M, 128)
            .transpose(3, 2, 0, 1, 4)
            .reshape(QKV_M, 128, 2 * KT * 128))

        # attn feature f: tile ft=f//128, partition p=f%128 -> g=p//64, head
        f = np.arange(1024)
        colsel = (16 * g2 + 8 * ((f % 128) // 64) + f // 128) * D + (f % 64)
        WoT = np.zeros((1024, OUT_M * 128), np.float32)
        WoT[:, :HID] = out_w[:, colsel].T * W_SCALE
        ohi, olo = _hilo(WoT)                        # (1024 features, OUT_M*128)
        wout = np.ascontiguousarray(
            np.stack([ohi, olo])                     # (2, 1024, 2944)
            .reshape(2, OUT_F, 128, OUT_M, 128)
            .transpose(3, 2, 0, 1, 4)                # [m, f-in-tile, hl, ftile, hid]
            .reshape(OUT_M, 128, 2 * OUT_F * 128))

        tp = np.zeros((HIDP, NT), np.float32)
        tp[:HID] = t_full[b].T * T_SCALE
        tp[HID] = T_SCALE                            # bias ones-row
        thi, tlo = _hilo(tp)
        t8 = np.ascontiguousarray(
            np.stack([thi, tlo]).reshape(2 * KT, 128, NT))

        sinkrow = np.empty((4, CH), np.float32)
        for g in range(2):
            for a in range(2):
                hl = 8 * g + 4 * a + np.arange(4)
                se = np.exp(sinks[16 * g2 + hl].astype(np.float32)
                            + np.float32(LOGIT_SHIFT)) / np.float32(A_SCALE)
                sinkrow[2 * g + a] = np.repeat(se, 128)

        in_maps.append({
            "t8": t8, "wqkv": wqkv, "wout": wout,
            "cos128": cos128, "sin128": sin128,
            "maskt": maskt, "identb": identb,
            "sinkrow": sinkrow, "ident64": ident64,
        })
    return in_maps


def unshard(results, x, out_b):
    x = np.asarray(x, np.float32)
    out_b = np.asarray(out_b, np.float32)
    y = np.empty((B, SEQ, HID), np.float32)
    for b in range(B):
        acc = np.zeros((OUT_M * 128, NT), np.float64)
        for g2 in range(4):
            acc += np.asarray(results[4 * b + g2]["out_t"]).astype(
                np.float64).reshape(OUT_M * 128, NT)
        y[b] = x[b] + acc[:HID].T.astype(np.float32) + out_b[None, :]
    return y


_NC_CACHE = []


def kernel(x, norm_w, qkv_w, qkv_b, out_w, out_b, sinks):
    in_maps = prep_inputs(x, norm_w, qkv_w, qkv_b, out_w, sinks)
    if not _NC_CACHE:
        _NC_CACHE.append(build_nc())
    nc = _NC_CACHE[0]
    res = run_bass_kernel_spmd(nc, in_maps, core_ids=list(range(8)))
    return unshard(res.results, x, out_b)
